# revision 31
# baseline (speedup 1.0000x reference)
"""Trainium2 Bass kernel for nn_MultiHeadAttention_26482768347194.

The reference softmaxes over a size-1 axis (all-ones attention), so the
module collapses exactly to

    z[b]     = (sum_l x[b,l,:]) @ (wv @ fc_w) + (L*bv @ fc_w + fc_b)
    out      = LayerNorm(x + z[:,None,:]) * ln_g + ln_b

v3 design (vs the 51-63us v2): v2 streamed 4MB of fp8 wv/fc weights and
was PE-bound on the matvec weight stream (75us of MATMUL+LDW).  By
associativity the two matvecs collapse into one 512x512 matrix
Wcomb = wv @ fc_w, precomputed on the host exactly like v2 precomputed
the bias path c.  That kills the weight stream entirely.

Per core (one batch element, data-parallel over 8 cores):
  ship:  x_g = x * ln_g     [128,8,512] bf16  (1MB;  g host-folded)
         xT  = x^T          [128,4,1024] fp8  (0.5MB; raw x, stats only)
         wq  = diag(1/g) Wcomb * 64  [4,128,512] fp8 (0.25MB)
         rows: c, g, b, misc.
  math:  gxsum = colsum(x_g) (PE, stream);  z = gxsum @ wq/256 + c
         per-token stats from xT via PE moving-form matmuls:
           m = ones.xT, sq = ones.xT^2 (Scalar squares), d = zc.xT
         rows -> one [6,512] psum bank -> one copy -> PE transposes
         -> col stats -> var_y = sq/D - (m/D)^2 + (2/D) d + var_z
         out_t = rstd_t * x_g_t + q3_t,
           q3_t = (-rstd*mu)xg + rstd (x) zc*g + 1 (x) b  (K=3 PE outer)
         tail split across DVE / GPSIMD (stt) to parallelize.

Numerics: fp8 e3m4 on xT touches only the stats/dot path (incoherent
~3% element error, /sqrt(512) after reduction); fp8 wcomb matches v2's
fp8 weight treatment; x_g/out are bf16/f16.  v2 measured 1.12e-2 with
two chained fp8 matmuls; this has one.

Self-contained: shapes hardcoded, no sibling imports.
"""

from contextlib import ExitStack

import numpy as np
import ml_dtypes

import concourse.bass as bass
import concourse.bacc as bacc
import concourse.mybir as mybir
import concourse.tile as tile
from concourse.bass_utils import run_bass_kernel_spmd

B, L, D, H = 8, 1024, 512, 8
HD = H * D
P = 128
NT = L // P          # 8 token tiles
KD = D // P          # 4 d-chunks
EPS = 1e-5
N_CORES = 8
WS = 64.0            # fp8 wcomb scale
INV_WS = 1.0 / WS

F32 = mybir.dt.float32
F16 = mybir.dt.float16
BF16 = mybir.dt.bfloat16
F8 = mybir.dt.float8e3
AF = mybir.ActivationFunctionType
ALU = mybir.AluOpType

# tail engine split per tile: 'd' = DVE stt; 'p' = PE diag-matmul
# (diag built by GPSIMD in SBUF, psum->obuf copy on Scalar)
TAIL = ['d', 'p', 'd', 'p', 'd', 'p', 'd', 'p']
# ssq square-pass split: Scalar does chunks [0, NSQ_S), DVE-ttr does tiles
# of the remaining d-range... keep simple: Scalar squares all 4 chunks.


def build_kernel():
    nc = bacc.Bacc("TRN2", target_bir_lowering=False, debug=False,
                   num_devices=N_CORES)

    # host-blocked layouts:
    #   xg[p, t, d]   = (x * g)[t*128 + p, d]            bf16
    #   xT[p, c, l]   = x[l, c*128 + p]                  fp8
    #   wq[c, p, n]   = (diag(1/g) wv fc_w)[c*128+p, n] * 64    fp8
    #   crow          = L*bv @ fc_w + fc_b               f32 [1, D]
    #   grow, brow    = ln_g, ln_b rows                  f32 [1, D]
    xg_d = nc.dram_tensor("xg", [P, NT, D], BF16, kind="ExternalInput")
    xT_d = nc.dram_tensor("xT", [P, KD, L], F8, kind="ExternalInput")
    wq_d = nc.dram_tensor("wq", [P, KD, D], F8, kind="ExternalInput")
    id_d = nc.dram_tensor("id128", [P, P], F32, kind="ExternalInput")
    gb3_d = nc.dram_tensor("gb3", [P, KD, 3], BF16, kind="ExternalInput")
    c_d = nc.dram_tensor("crow", [1, D], F32, kind="ExternalInput")
    out_d = nc.dram_tensor("out", [L, D], F16, kind="ExternalOutput")
    import os
    dbg = os.environ.get("KERNEL_DEBUG_TAPS") == "1"
    if dbg:
        dbg_z = nc.dram_tensor("dbg_z", [1, D], F32, kind="ExternalOutput")
        dbg_st = nc.dram_tensor("dbg_st", [P, 24], F32, kind="ExternalOutput")
        dbg_var = nc.dram_tensor("dbg_var", [P, NT], F32,
                                 kind="ExternalOutput")

    out_v = out_d.ap().rearrange("(t p) d -> p t d", p=P)

    with tile.TileContext(nc, pool_alloc_mode="queue") as tc, \
            ExitStack() as ctx:
        ctx.enter_context(nc.allow_low_precision(
            reason="bf16/fp8 stats paths validated end-to-end"))
        consts = ctx.enter_context(tc.tile_pool(name="consts", bufs=1))
        work = ctx.enter_context(tc.tile_pool(name="work", bufs=3))
        psum = ctx.enter_context(
            tc.tile_pool(name="psum", bufs=1, space=bass.MemorySpace.PSUM))

        # ---- tiny SBUF constants ---------------------------------------
        ones_col = consts.tile([P, 1], BF16)
        nc.gpsimd.memset(ones_col[:], 1.0)
        ones_row = consts.tile([1, P], BF16)
        nc.gpsimd.memset(ones_row[:], 1.0)
        eps_t = consts.tile([P, 1], F32)
        nc.gpsimd.memset(eps_t[:], EPS)
        # warm Scalar's table with the function set containing Square+Sqrt
        warm = consts.tile([P, 1], F32)
        nc.scalar.activation(warm[:], eps_t[:], AF.Square)

        # ---- DMA program ----------------------------------------------
        # xT first (feeds the Scalar square pipeline), then x_g (feeds
        # PE colsum + tail), weights + rows behind.
        xT_t = consts.tile([P, KD, L], F8)
        xg_t = consts.tile([P, NT, D], BF16)
        wq_t = consts.tile([P, KD, D], F8)
        id_t = consts.tile([P, P], F32)
        asm3 = consts.tile([P, KD, 3], BF16)   # cols [g | (z*g) | b]
        c_t = consts.tile([1, D], F32)
        # spread trigger issue across engines so all queues fill early
        nc.sync.dma_start(xT_t[:], xT_d.ap())
        nc.scalar.dma_start(xg_t[:, 0:4, :], xg_d.ap()[:, 0:4, :])
        nc.sync.dma_start(xg_t[:, 4:8, :], xg_d.ap()[:, 4:8, :])
        nc.gpsimd.dma_start(wq_t[:], wq_d.ap())
        nc.gpsimd.dma_start(id_t[:], id_d.ap())
        nc.gpsimd.dma_start(asm3[:], gb3_d.ap())
        nc.gpsimd.dma_start(c_t[:], c_d.ap())
        id_b = consts.tile([P, P], BF16)
        nc.scalar.activation(id_b[:], id_t[:], AF.Identity)

        # PE p-state warm-up: a long unbroken chain of dummy matmuls on
        # SBUF constants (no data deps) ramps the PE clock to full speed
        # while the input DMA is in flight.
        warm_rhs = consts.tile([P, 256], BF16)
        nc.gpsimd.memset(warm_rhs[:], 0.0)
        ps_warm = psum.tile([1, 256], F32, tag="bigbank", bufs=2)
        for _ in range(30):
            nc.tensor.matmul(ps_warm[:], ones_col[:], warm_rhs[:],
                             start=True, stop=True)

        # ---- stream phase ----------------------------------------------
        # stat-row psum: bankA partitions 0:2 = (m,d) half0, 32:34 = (m,d)
        # half1, 64:65 = sq half0; bankB 0:1 = sq half1.  (matmul outputs
        # must start at partition 0/32/64.)
        ps_rowsA = psum.tile([66, 512], F32, tag="rowsA", bufs=1)
        ps_rowsB = psum.tile([1, 512], F32, tag="rowsB", bufs=1)

        # lhsT2 per chunk: [-1/D | 2z/D] interleaved in [P, 2*KD]
        l2 = consts.tile([P, 2 * KD], BF16)
        nc.gpsimd.memset(l2[:, 0:2 * KD:2], -1.0 / D)
        invD_col = consts.tile([P, 1], BF16)
        nc.gpsimd.memset(invD_col[:], 1.0 / D)
        half_col = consts.tile([P, 1], BF16)
        nc.gpsimd.memset(half_col[:], 0.5)
        d4_col = consts.tile([P, 1], BF16)
        nc.gpsimd.memset(d4_col[:], float(D) / 4.0)

        # Scalar: squares xT chunk-by-chunk into xsqT (bf16)
        xsq_t = consts.tile([P, KD, L], BF16)
        # PE: gxsum row accumulation from x_g tiles
        ps_gxs = psum.tile([1, D], F32, tag="gxs", bufs=1)

        for c in range(KD):
            nc.scalar.activation(xsq_t[:, c, :], xT_t[:, c, :], AF.Square)

        for t in range(NT):
            nc.tensor.matmul(ps_gxs[:], ones_col[:], xg_t[:, t, :],
                             start=(t == 0), stop=(t == NT - 1))

        # PE: sq rows (ones . xT^2) per half of L (stream phase)
        for c in range(KD):
            nc.tensor.matmul(
                ps_rowsA[64:65, :], invD_col[:], xsq_t[:, c, 0:512],
                start=(c == 0), stop=(c == KD - 1))
        for c in range(KD):
            nc.tensor.matmul(
                ps_rowsB[0:1, :], invD_col[:], xsq_t[:, c, 512:1024],
                start=(c == 0), stop=(c == KD - 1))

        # ---- z chain ---------------------------------------------------
        # gxsum row -> SBUF bf16 -> 4 col transposes -> matvec -> z row
        gxs_row = consts.tile([1, D], BF16)
        nc.scalar.activation(gxs_row[:], ps_gxs[:], AF.Identity)
        ps_xsT = psum.tile([P, 2 * KD], BF16, tag="xsT", bufs=1)
        for c in range(KD):
            nc.tensor.transpose(ps_xsT[:, 2 * c:2 * c + 1],
                                gxs_row[0:1, c * P:(c + 1) * P],
                                id_b[0:1, 0:1])
        xsT = consts.tile([P, KD], BF16)
        nc.scalar.activation(xsT[:], ps_xsT[:, 0:2 * KD:2], AF.Identity)

        ps_z = psum.tile([1, D], F32, tag="gxs", bufs=1)
        for c in range(KD):
            nc.tensor.matmul(ps_z[:], xsT[:, c:c + 1], wq_t[:, c, :],
                             start=(c == 0), stop=(c == KD - 1))
        # z = ps_z * (1/WS) + c, straight to bf16 row
        z_rowb = consts.tile([1, D], BF16)
        nc.vector.scalar_tensor_tensor(z_rowb[:], ps_z[:], INV_WS, c_t[:],
                                       op0=ALU.mult, op1=ALU.add)

        # raw z cols via transposes; feed l2 (scaled 2/D), asm3 (z*g),
        # and the S1/S2 moment matmuls
        ps_zcc = psum.tile([P, 2 * KD], BF16, tag="xsT", bufs=1)
        for c in range(KD):
            nc.tensor.transpose(ps_zcc[:, 2 * c:2 * c + 1],
                                z_rowb[0:1, c * P:(c + 1) * P],
                                id_b[0:1, 0:1])
        nc.scalar.activation(l2[:, 1:2 * KD:2], ps_zcc[:, 0:2 * KD:2],
                             AF.Identity, scale=2.0 / D)
        nc.vector.scalar_tensor_tensor(
            asm3[:, :, 1], l2[:, 1:2 * KD:2], float(D) / 2.0,
            asm3[:, :, 0], op0=ALU.mult, op1=ALU.mult)

        # assemble rhs3 rows [g; z*g; b] by transposing asm3 chunks
        ps_r3 = psum.tile([3, D], BF16, tag="xsT", bufs=1)
        for c in range(KD):
            nc.tensor.transpose(ps_r3[:, c * P:(c + 1) * P],
                                asm3[:, c, :], id_b[:])
        rhs3 = consts.tile([3, D], BF16)
        nc.scalar.activation(rhs3[:], ps_r3[:], AF.Identity)

        # scalar moments of z: S1 = sum(z)/D, S2 = sum(z^2)/D, via the
        # (2z/D) cols in l2: S1 = 0.5*sum(zc2), S2 = (D/4)*sum(zc2^2)
        zsq_c = consts.tile([P, KD], BF16)
        nc.scalar.activation(zsq_c[:], l2[:, 1:2 * KD:2], AF.Square)
        ps_s = psum.tile([1, 2 * KD], F32, tag="gxs", bufs=1)
        nc.tensor.matmul(ps_s[:, 0:KD], half_col[:], l2[:, 1:2 * KD:2],
                         start=True, stop=True)
        nc.tensor.matmul(ps_s[:, KD:2 * KD], d4_col[:], zsq_c[:],
                         start=True, stop=True)
        s12_row = consts.tile([1, 2], F32)
        nc.vector.tensor_reduce(s12_row[:, 0:1], ps_s[:, 0:KD],
                                axis=mybir.AxisListType.X, op=ALU.add)
        nc.vector.tensor_reduce(s12_row[:, 1:2], ps_s[:, KD:2 * KD],
                                axis=mybir.AxisListType.X, op=ALU.add)
        s12_b = consts.tile([1, 2], BF16)
        nc.vector.tensor_copy(s12_b[:], s12_row[:])
        # broadcast S1,S2 to all partitions
        ps_sb = psum.tile([P, 2], F32, tag="stT", bufs=1)
        nc.tensor.matmul(ps_sb[:], ones_row[:], s12_b[:],
                         start=True, stop=True)
        sb_s = consts.tile([P, 2], F32)
        nc.scalar.activation(sb_s[:], ps_sb[:], AF.Identity)
        # cs cols: [0]=-2*S1, [1]=-S1, [2]=S2-S1^2+eps
        cs = consts.tile([P, 3], F32)
        nc.vector.tensor_scalar(cs[:, 0:1], sb_s[:, 0:1], -2.0, None,
                                op0=ALU.mult)
        nc.vector.tensor_scalar(cs[:, 1:2], sb_s[:, 0:1], -1.0, None,
                                op0=ALU.mult)
        t_s1sq = work.tile([P, 1], F32, tag="s1sq")
        nc.vector.scalar_tensor_tensor(t_s1sq[:], sb_s[:, 0:1], -1.0,
                                       sb_s[:, 0:1],
                                       op0=ALU.mult, op1=ALU.mult)
        nc.vector.scalar_tensor_tensor(cs[:, 2:3], sb_s[:, 1:2], EPS,
                                       t_s1sq[:], op0=ALU.add, op1=ALU.add)

        # (m, d) rows per half: lhsT2 . xT chunk, M=2
        for c in range(KD):
            nc.tensor.matmul(
                ps_rowsA[0:2, :], l2[:, 2 * c:2 * c + 2],
                xT_t[:, c, 0:512],
                start=(c == 0), stop=(c == KD - 1))
        for c in range(KD):
            nc.tensor.matmul(
                ps_rowsA[32:34, :], l2[:, 2 * c:2 * c + 2],
                xT_t[:, c, 512:1024],
                start=(c == 0), stop=(c == KD - 1))

        # ---- stats: rows -> cols ---------------------------------------
        statrows = consts.tile([66, 512], F32)
        nc.vector.tensor_copy(statrows[0:2, :], ps_rowsA[0:2, :])
        nc.scalar.activation(statrows[32:34, :], ps_rowsA[32:34, :],
                             AF.Identity)
        nc.scalar.activation(statrows[64:65, :], ps_rowsA[64:65, :],
                             AF.Identity)
        statrowsB = consts.tile([1, 512], F32)
        nc.vector.tensor_copy(statrowsB[:], ps_rowsB[:])
        # per tile t: half h=t//4, slice s=t%4; transpose the three
        # [1,128] stat rows (m, sq, d) for that token range -> cols
        ps_st = psum.tile([P, 3 * NT], F32, tag="stT", bufs=1)
        for t in range(NT):
            h, s = t // 4, t % 4
            sl = slice(s * P, (s + 1) * P)
            md_rows = statrows[32 * h:32 * h + 2, sl]
            md_id = (id_t[0:2, 0:2] if h == 0
                     else id_t[32:34, 32:34])
            sq_row = (statrows[64:65, sl] if h == 0
                      else statrowsB[0:1, sl])
            sq_id = (id_t[64:65, 64:65] if h == 0
                     else id_t[0:1, 0:1])
            nc.tensor.transpose(ps_st[:, 3 * t:3 * t + 2], md_rows, md_id)
            nc.tensor.transpose(ps_st[:, 3 * t + 2:3 * t + 3], sq_row,
                                sq_id)
        stc = consts.tile([P, 3 * NT], F32)
        nc.scalar.activation(stc[:], ps_st[:], AF.Identity)
        m8 = stc[:, 0:3 * NT:3]
        d8 = stc[:, 1:3 * NT:3]
        sq8 = stc[:, 2:3 * NT:3]

        # m8 = -mu (lhsT was -1/D), sq8 = sum(x^2)/D, d8 = (2/D)sum(xz)
        # var8 = sq8 - mu^2 - 2 mu S1 + d8 + (S2 - S1^2 + eps)
        #      = sq8 - m8*(m8 + (-2 S1)*(-1))... using negmu:
        #   v1 = (m8 + cs0) * m8  = mu^2 + 2 mu S1
        msq = work.tile([P, NT], F32, tag="msq")
        nc.vector.scalar_tensor_tensor(msq[:], m8, cs[:, 0:1], m8,
                                       op0=ALU.add, op1=ALU.mult)
        c8 = work.tile([P, NT], F32, tag="c8")
        nc.vector.tensor_tensor(c8[:], sq8, msq[:], ALU.subtract)
        var8 = consts.tile([P, NT], F32)
        nc.vector.scalar_tensor_tensor(var8[:], d8, cs[:, 2:3], c8[:],
                                       op0=ALU.add, op1=ALU.add)
        std8 = consts.tile([P, NT], F32)
        nc.scalar.activation(std8[:], var8[:], AF.Sqrt)
        rstd8 = consts.tile([P, NT], BF16)
        nc.vector.reciprocal(rstd8[:], std8[:])
        rstd8f = consts.tile([P, NT], F32)
        nc.vector.tensor_copy(rstd8f[:], rstd8[:])

        if dbg:
            dz = consts.tile([1, D], F32)
            nc.vector.tensor_copy(dz[:], z_row[:])
            nc.sync.dma_start(dbg_z.ap(), dz[:])
            dst = consts.tile([P, 24], F32)
            nc.vector.tensor_copy(dst[:], stc[:])
            nc.sync.dma_start(dbg_st.ap(), dst[:])
            dvar = consts.tile([P, NT], F32)
            nc.vector.tensor_copy(dvar[:], var8[:])
            nc.sync.dma_start(dbg_var.ap(), dvar[:])

        # nm8 = (negmu - S1) * rstd, interleaved [nm|rstd|ones] [P, 24]
        nr24 = consts.tile([P, 3 * NT], BF16)
        nc.gpsimd.memset(nr24[:, 2:3 * NT:3], 1.0)
        nc.vector.scalar_tensor_tensor(nr24[:, 0:3 * NT:3], m8,
                                       cs[:, 1:2], rstd8f[:],
                                       op0=ALU.add, op1=ALU.mult)
        nc.vector.tensor_copy(nr24[:, 1:3 * NT:3], rstd8[:])

        # ---- tail ------------------------------------------------------
        obuf = consts.tile([P, NT, D], F16)
        # batch all nr transposes, l3 copies, and diag builds up front
        ps_nr = psum.tile([3, P * NT], BF16, tag="nr", bufs=1)
        for t in range(NT):
            nc.tensor.transpose(ps_nr[:, t * P:(t + 1) * P],
                                nr24[:, 3 * t:3 * t + 3], id_b[:])
        lhsT3s = []
        for t in range(NT):
            l3 = consts.tile([3, P], BF16, tag="l3", bufs=NT)
            lhsT3s.append(l3)
            if t % 2 == 0:
                nc.vector.tensor_copy(l3[:], ps_nr[:, t * P:(t + 1) * P])
            else:
                nc.scalar.activation(l3[:], ps_nr[:, t * P:(t + 1) * P],
                                     AF.Identity)
        diags = {}
        for t in range(NT):
            if TAIL[t] == 'p':
                dg = work.tile([P, P], BF16, tag="diag", bufs=4)
                nc.vector.tensor_scalar_mul(dg[:], id_b[:],
                                            rstd8f[:, t:t + 1])
                diags[t] = dg

        for t in range(NT):
            ps_q = psum.tile([P, D], F32, tag="bigbank", bufs=2)
            if TAIL[t] == 'd':
                nc.tensor.matmul(ps_q[:], lhsT3s[t][:], rhs3[:],
                                 start=True, stop=True)
                nc.vector.scalar_tensor_tensor(
                    obuf[:, t, :], xg_t[:, t, :], rstd8[:, t:t + 1],
                    ps_q[:], op0=ALU.mult, op1=ALU.add)
            else:
                nc.tensor.matmul(ps_q[:], diags[t][:], xg_t[:, t, :],
                                 start=True, stop=False)
                nc.tensor.matmul(ps_q[:], lhsT3s[t][:], rhs3[:],
                                 start=False, stop=True)
                nc.scalar.activation(obuf[:, t, :], ps_q[:], AF.Identity)
            trig = nc.gpsimd if t % 2 == 0 else nc.sync
            trig.dma_start(out_v[:, t:t + 1, :], obuf[:, t:t + 1, :])

    nc.compile()
    return nc


_NC_CACHE = None


def _get_nc():
    global _NC_CACHE
    if _NC_CACHE is None:
        _NC_CACHE = build_kernel()
    return _NC_CACHE


def _shard_inputs(inputs):
    bf = ml_dtypes.bfloat16
    f8 = ml_dtypes.float8_e3m4
    x = np.asarray(inputs["input"], dtype=np.float32)
    wv = np.asarray(inputs["wv"], dtype=np.float32)
    bv = np.asarray(inputs["bv"], dtype=np.float32)
    fc_w = np.asarray(inputs["fc_w"], dtype=np.float32)
    fc_b = np.asarray(inputs["fc_b"], dtype=np.float32)
    ln_g = np.asarray(inputs["ln_g"], dtype=np.float32)
    ln_b = np.asarray(inputs["ln_b"], dtype=np.float32)

    wcomb = (wv @ fc_w) / ln_g[:, None]          # diag(1/g) @ (wv @ fc_w)
    wq = (wcomb * WS).astype(f8)
    wq_bl = np.ascontiguousarray(
        wq.reshape(KD, P, D).transpose(1, 0, 2))       # [p, c, n]

    c_vec = (float(L) * bv) @ fc_w + fc_b
    crow = np.ascontiguousarray(c_vec[None, :])
    gb3 = np.zeros((P, KD, 3), dtype=np.float32)
    gb3[:, :, 0] = ln_g.reshape(KD, P).T
    gb3[:, :, 2] = ln_b.reshape(KD, P).T
    gb3 = gb3.astype(bf)
    id128 = np.eye(P, dtype=np.float32)

    in_maps = []
    for i in range(N_CORES):
        xgv = (x[i] * ln_g[None, :])
        xg_bl = np.ascontiguousarray(
            xgv.reshape(NT, P, D).transpose(1, 0, 2)).astype(bf)
        xT_bl = np.ascontiguousarray(
            x[i].T.reshape(KD, P, L).transpose(1, 0, 2)).astype(f8)
        in_maps.append({
            "xg": xg_bl,
            "xT": xT_bl,
            "wq": wq_bl,
            "id128": id128,
            "gb3": gb3,
            "crow": crow,
        })
    return in_maps


def kernel(**inputs) -> np.ndarray:
    nc = _get_nc()
    in_maps = _shard_inputs(inputs)
    res = run_bass_kernel_spmd(nc, in_maps, core_ids=list(range(N_CORES)))
    out = np.stack([res.results[i]["out"] for i in range(N_CORES)], axis=0)
    return out.astype(np.float32)


def _install_ntff_hook_shim():
    import sys
    import types
    try:
        from antenv.axon_hooks import get_axon_ntff_profile_hook  # noqa: F401
        return
    except ImportError:
        pass
    try:
        from trn_agent_boot.trn_boot import _ntff_profile_via_ctypes
        hook = _ntff_profile_via_ctypes("/opt/axon/libaxon_pjrt.so")
    except Exception:
        hook = None
    mod = types.ModuleType("antenv.axon_hooks")
    state = {"hook": hook}
    mod.get_axon_ntff_profile_hook = lambda: state["hook"]
    mod.set_axon_ntff_profile_hook = lambda h: state.update(hook=h)
    sys.modules["antenv.axon_hooks"] = mod
    import antenv
    antenv.axon_hooks = mod


def kernel_profiled(inputs, trace_cores=None):
    _install_ntff_hook_shim()
    nc = _get_nc()
    in_maps = _shard_inputs(inputs)
    res = run_bass_kernel_spmd(
        nc, in_maps, core_ids=list(range(N_CORES)), trace=True,
        trace_cores=trace_cores if trace_cores is not None else [0])
    out = np.stack([res.results[i]["out"] for i in range(N_CORES)], axis=0)
    return out.astype(np.float32), res


def _ref_one(x, wv, bv, fc_w, fc_b, g, b):
    xsum = x.sum(0)
    z = (xsum @ wv + L * bv) @ fc_w + fc_b
    y = x + z[None, :]
    mu = y.mean(-1, keepdims=True)
    var = y.var(-1, keepdims=True)
    return (y - mu) / np.sqrt(var + EPS) * g + b


if __name__ == "__main__":
    import sys
    if "--sim" in sys.argv:
        from concourse.bass_interp import CoreSim
        rng = np.random.default_rng(0)
        x = rng.standard_normal((B, L, D), dtype=np.float32)
        wv = (rng.uniform(-1, 1, (D, HD)) / np.sqrt(D)).astype(np.float32)
        bv = (rng.uniform(-1, 1, HD) / np.sqrt(D)).astype(np.float32)
        fc_w = (rng.uniform(-1, 1, (HD, D)) / np.sqrt(HD)).astype(np.float32)
        fc_b = (rng.uniform(-1, 1, D) / np.sqrt(HD)).astype(np.float32)
        g = np.ones(D, dtype=np.float32)
        b = np.zeros(D, dtype=np.float32)
        inputs = dict(input=x, wv=wv, bv=bv, fc_w=fc_w, fc_b=fc_b,
                      ln_g=g, ln_b=b)

        nc = _get_nc()
        in_maps = _shard_inputs(inputs)
        sim = CoreSim(nc, trace=False)
        for k, v in in_maps[0].items():
            sim.tensor(k)[:] = v
        sim.simulate()
        got = np.array(sim.tensor("out")).astype(np.float32)

        want = _ref_one(x[0], wv, bv, fc_w, fc_b, g, b)
        err = np.abs(got - want).max() / np.abs(want).max()
        print("sim absmax rel err:", err)
        assert err < 2e-2, err
        print("SIM PASS")
